# revision 1
# baseline (speedup 1.0000x reference)
"""Trainium2 Bass kernel for nn_Decoder_22703197127089 (moe_routing).

Key insight: the module's output depends only on each sample's LAST token
(h[:, -1, :] is taken after the MoE block), so the MoE block and all
attention rows except the last are dead code.  What remains per sample:
  conv1d patch embed (all 1023 tokens)  -> LN1 -> k,v projections (all
  tokens) + q for the last token -> one attention row -> out-proj ->
  MoE for 1 token -> LN2 -> final linear (96).

Sharding: data-parallel over batch B=32 across 8 cores (4 samples/core).
No collectives; host gathers the (4, 96) per-core outputs.

Layouts on device (per core):
  - X2 tile (128, L): partitions 0-63 = X[c, l], 64-127 = X[c, l+1]
    so that conv contraction chunks of 128 = (p in {2k, 2k+1}) x (c in
    0..63) are strided APs (offset 2k, stride 12 over patches).
  - conv output h0T is feature-major (64, N) per sample; two samples of
    a pair live stacked in one (128, N) tile (col-tiled matmul pair).
  - LN1 is folded into the projections:
      k = LN(h0) @ kw.T  =>  scores use k0 = kw @ h0T plus per-token
      (mu, rstd) corrections; v = LN(h0) @ vw.T folded into the
      attention weighted sum.  Per-token stats come from PE matmuls
      against a [ones;0]/[0;ones] selector (cross-partition reduce).
"""

import numpy as np

import concourse.bass as bass
import concourse.mybir as mybir
import concourse.tile as tile
from concourse import bacc
from concourse.bass_utils import run_bass_kernel_spmd

F32 = mybir.dt.float32
AF = mybir.ActivationFunctionType
OP = mybir.AluOpType

B, C, L = 32, 64, 12288
D = 64
E = 8
TOPK = 4
P, S = 24, 12
PRED = 96
N = (L - P) // S + 1  # 1023
NCORES = 8
SPC = B // NCORES     # 4 samples per core
NPAIR = SPC // 2      # 2
NCH = (C * P) // 128  # 12 contraction chunks of K=128 (p-pair, c)
NT = 1024             # padded token dim (col 1023 zeroed)
NJ = 8                # 128-token chunks
EPS = 1e-5
MCS = (512, 511)      # conv/k0 m-chunk sizes
DT_X = mybir.dt.bfloat16  # X/conv-weight compute dtype (F32 for exact)


def _pos_encoding_np(n, d):
    # match reference._pos_encoding in float32
    pos = np.arange(n, dtype=np.float32)[:, None]
    div = np.exp(np.arange(0, d, 2, dtype=np.float32)
                 * (np.float32(-np.log(np.float32(10000.0))) / np.float32(d)))
    pe = np.zeros((n, d), np.float32)
    pe[:, 0::2] = np.sin(pos * div)
    pe[:, 1::2] = np.cos(pos * div)
    return pe


def build_nc(debug_taps=False):
    nc = bacc.Bacc("TRN2", target_bir_lowering=False, debug=False,
                   num_devices=NCORES)

    inp = {}
    def di(name, shape, dtype=F32):
        inp[name] = nc.dram_tensor(name, list(shape), dtype,
                                   kind="ExternalInput")
        return inp[name]

    Xs = di("Xs", (SPC, C, L), DT_X)
    Wc = di("Wc", (C * P, D), DT_X)       # (p,c)-ordered conv weight
    PEBT2 = di("PEBT2", (128, N))          # [pebT; pebT]
    QwT2 = di("QwT2", (128, D))
    Kw2 = di("Kw2", (128, D))
    VwT2 = di("VwT2", (128, D))
    OwT = di("OwT", (D, D))                # ow.T
    SqCol2 = di("SqCol2", (128, 1))        # [qw.sum(1); qw.sum(1)]
    SkCol2 = di("SkCol2", (128, 1))
    SvCol = di("SvCol", (D, 1))
    SelAB = di("SelAB", (128, 2))          # [[1;0],[0;1]] selector
    OnesRow = di("OnesRow", (1, 128))
    Ones128 = di("Ones128", (128, 1))
    OneHot = di("OneHot", (128, 1))        # 1.0 at partition 126
    LastMask = di("LastMask", (128, 1))    # ones, 0.0 at partition 127
    RwT = di("RwT", (D, E))
    WexpE = di("WexpE", (D + 1, E * D))    # experts + bias row
    MowT = di("MowT", (D, D))
    OutWT = di("OutWT", (D, PRED))
    Id4 = di("Id4", (SPC, SPC))

    Yout = nc.dram_tensor("Yout", [SPC, PRED], F32, kind="ExternalOutput")
    taps = {}

    with tile.TileContext(nc) as tc:
        with (
            tc.tile_pool(name="const", bufs=1) as pc,
            tc.tile_pool(name="xp", bufs=4) as xp,
            tc.tile_pool(name="hp", bufs=2) as hp,
            tc.tile_pool(name="sqp", bufs=2) as sqp,
            tc.tile_pool(name="vp", bufs=2) as vp,
            tc.tile_pool(name="sm", bufs=2) as sm,
            tc.tile_pool(name="ps", bufs=1, space="PSUM") as ps,
        ):
            # ---- constants to SBUF ----
            wsb = pc.tile([128, NCH * D], DT_X, tag="wsb")
            nc.gpsimd.dma_start(
                wsb[:].rearrange("p (k d) -> p k d", k=NCH),
                Wc.ap().rearrange("(k p) d -> p k d", p=128))
            pebt = pc.tile([128, N], F32, tag="pebt")
            nc.gpsimd.dma_start(pebt[:], PEBT2.ap())
            qwt = pc.tile([128, D], F32, tag="qwt")
            nc.gpsimd.dma_start(qwt[:], QwT2.ap())
            kw2 = pc.tile([128, D], F32, tag="kw2")
            nc.gpsimd.dma_start(kw2[:], Kw2.ap())
            vwt = pc.tile([128, D], F32, tag="vwt")
            nc.gpsimd.dma_start(vwt[:], VwT2.ap())
            owt = pc.tile([D, D], F32, tag="owt")
            nc.gpsimd.dma_start(owt[:], OwT.ap())
            sqcol = pc.tile([128, 1], F32, tag="sqcol")
            nc.gpsimd.dma_start(sqcol[:], SqCol2.ap())
            skcol = pc.tile([128, 1], F32, tag="skcol")
            nc.gpsimd.dma_start(skcol[:], SkCol2.ap())
            svcol = pc.tile([D, 1], F32, tag="svcol")
            nc.gpsimd.dma_start(svcol[:], SvCol.ap())
            selab = pc.tile([128, 2], F32, tag="selab")
            nc.gpsimd.dma_start(selab[:], SelAB.ap())
            onesr = pc.tile([1, 128], F32, tag="onesr")
            nc.gpsimd.dma_start(onesr[:], OnesRow.ap())
            ones128 = pc.tile([128, 1], F32, tag="ones128")
            nc.gpsimd.dma_start(ones128[:], Ones128.ap())
            onehot = pc.tile([128, 1], F32, tag="onehot")
            nc.gpsimd.dma_start(onehot[:], OneHot.ap())
            lastm = pc.tile([128, 1], F32, tag="lastm")
            nc.gpsimd.dma_start(lastm[:], LastMask.ap())
            rwt = pc.tile([D, E], F32, tag="rwt")
            nc.gpsimd.dma_start(rwt[:], RwT.ap())
            wexp = pc.tile([D + 1, E * D], F32, tag="wexp")
            nc.gpsimd.dma_start(wexp[:], WexpE.ap())
            mowt = pc.tile([D, D], F32, tag="mowt")
            nc.gpsimd.dma_start(mowt[:], MowT.ap())
            outwt = pc.tile([D, PRED], F32, tag="outwt")
            nc.gpsimd.dma_start(outwt[:], OutWT.ap())
            id4 = pc.tile([SPC, SPC], F32, tag="id4")
            nc.gpsimd.dma_start(id4[:], Id4.ap())

            # attention outputs of all samples, + ones row for expert bias
            ha = pc.tile([D + 1, SPC], F32, tag="ha")
            nc.vector.memset(ha[D:D + 1, :], 1.0)
            epsb = pc.tile([128, 1], F32, tag="epsb")
            nc.vector.memset(epsb[:], EPS)

            for pair in range(NPAIR):
                # ---- X load: per-sample tile, partitions 64-127 hold
                # X shifted left by one so K=128 chunks cover (p, p+1) pairs
                XSPLIT = 6656
                x2 = []
                for s01 in range(2):
                    t = xp.tile([128, L], DT_X, tag="x2", name="x2t")
                    nc.sync.dma_start(t[0:C, 0:XSPLIT],
                                      Xs.ap()[2 * pair + s01][:, 0:XSPLIT])
                    nc.sync.dma_start(t[0:C, XSPLIT:L],
                                      Xs.ap()[2 * pair + s01][:, XSPLIT:L])
                    nc.sync.dma_start(t[C:128, 0:XSPLIT - 1],
                                      t[0:C, 1:XSPLIT])
                    nc.sync.dma_start(t[C:128, XSPLIT - 1:L - 1],
                                      t[0:C, XSPLIT:L])
                    x2.append(t)

                # ---- conv -> h0T pair (feature-major, A rows 0-63 / B 64-127)
                h0 = hp.tile([128, NT], F32, tag="h0")
                nc.vector.memset(h0[:, N:NT], 0.0)
                n0 = 0
                xv = [t[:].rearrange("p (n s) -> p n s", s=S) for t in x2]
                for mc, nn in enumerate(MCS):
                    cps = ps.tile([128, 512], F32, tag="convA", name="cps", bufs=2)
                    for k in range(NCH):
                        q, r = divmod(2 * k, S)
                        for s01 in range(2):
                            o = 64 * s01
                            nc.tensor.matmul(
                                cps[o:o + 64, 0:nn],
                                lhsT=wsb[:, D * k:D * k + D],
                                rhs=xv[s01][:, n0 + q:n0 + q + nn, r],
                                start=(k == 0), stop=(k == NCH - 1))
                    nc.vector.tensor_add(h0[:, n0:n0 + nn], cps[:, 0:nn],
                                         pebt[:, n0:n0 + nn])
                    n0 += nn

                # ---- LN1 stats: per-token colsum & sum-of-squares ----
                sq = sqp.tile([128, NT], F32, tag="sq")
                nc.scalar.activation(sq[:], h0[:], AF.Square)
                stp = ps.tile([128, 32], F32, tag="stats", bufs=2)
                for j in range(NJ):
                    nc.tensor.matmul(stp[:, 2 * j:2 * j + 2],
                                     lhsT=h0[:, 128 * j:128 * j + 128],
                                     rhs=selab[:], start=True, stop=True)
                    nc.tensor.matmul(stp[:, 16 + 2 * j:16 + 2 * j + 2],
                                     lhsT=sq[:, 128 * j:128 * j + 128],
                                     rhs=selab[:], start=True, stop=True)
                mean = sm.tile([128, 16], F32, tag="mean")
                nc.vector.tensor_scalar_mul(mean[:], stp[:, 0:16], 1.0 / D)
                ex2 = sm.tile([128, 16], F32, tag="ex2")
                nc.vector.tensor_scalar_mul(ex2[:], stp[:, 16:32], 1.0 / D)
                var = sm.tile([128, 16], F32, tag="var")
                nc.vector.tensor_mul(var[:], mean[:], mean[:])
                nc.vector.tensor_sub(var[:], ex2[:], var[:])
                std = sm.tile([128, 16], F32, tag="std")
                nc.scalar.activation(std[:], var[:], AF.Sqrt, bias=epsb[:])
                rstd = sm.tile([128, 16], F32, tag="rstd")
                nc.vector.reciprocal(rstd[:], std[:])
                r8 = sm.tile([128, 16], F32, tag="r8")
                nc.vector.tensor_scalar_mul(r8[:], rstd[:], 0.125)

                # ---- attention row, both samples of the pair packed ----
                # misc psum column map:
                #  0 q0(A rows 0:64 / B rows 64:128); 2-3 cb pair; 4-19 scores
                #  (col 4+2j+t); 20-23 extracts muA,rA,muB,rB; 24 mu bcast,
                #  25 r bcast (per-sample rows); 26-27 c1 A/B; 28 z2(2x1);
                #  29-30 zrow(1x2); 32-33 rzb(128x2); 34 g2; 35-36 grow;
                #  37-38 gb(64x2); 39-40 att A/B; 41-42 proj A/B
                misc = ps.tile([128, 44], F32, tag="misc", bufs=2)
                qe = sm.tile([128, 1], F32, tag="qe")
                for s01 in range(2):
                    o = 64 * s01
                    nc.tensor.matmul(misc[o:o + 64, 0:1], lhsT=qwt[o:o + 64, :],
                                     rhs=h0[o:o + 64, N - 1:N],
                                     start=True, stop=True)
                    nc.tensor.matmul(misc[0:1, 20 + 2 * s01:21 + 2 * s01],
                                     lhsT=mean[:, 14 + s01:15 + s01],
                                     rhs=onehot[:], start=True, stop=True)
                    nc.tensor.matmul(misc[0:1, 21 + 2 * s01:22 + 2 * s01],
                                     lhsT=rstd[:, 14 + s01:15 + s01],
                                     rhs=onehot[:], start=True, stop=True)
                ex4 = sm.tile([1, 4], F32, tag="ex4")
                nc.vector.tensor_copy(ex4[:], misc[0:1, 20:24])
                for s01 in range(2):
                    o = 64 * s01
                    # broadcast mu_last, r_last to this sample's 64 partitions
                    nc.tensor.matmul(misc[o:o + 64, 24:25],
                                     lhsT=onesr[0:1, 0:64],
                                     rhs=ex4[0:1, 2 * s01:2 * s01 + 1],
                                     start=True, stop=True)
                    nc.tensor.matmul(misc[o:o + 64, 25:26],
                                     lhsT=onesr[0:1, 0:64],
                                     rhs=ex4[0:1, 2 * s01 + 1:2 * s01 + 2],
                                     start=True, stop=True)
                    # q_eff = r_last * (q0 - mu_last * Sq)
                    nc.vector.tensor_mul(qe[o:o + 64, :], sqcol[o:o + 64, :],
                                         misc[o:o + 64, 24:25])
                    nc.vector.tensor_sub(qe[o:o + 64, :], misc[o:o + 64, 0:1],
                                         qe[o:o + 64, :])
                    nc.vector.tensor_mul(qe[o:o + 64, :], qe[o:o + 64, :],
                                         misc[o:o + 64, 25:26])
                    nc.tensor.matmul(misc[0:1, 26 + s01:27 + s01],
                                     lhsT=qe[o:o + 64, :],
                                     rhs=skcol[o:o + 64, :],
                                     start=True, stop=True)
                    # qk = kw.T @ q_eff: scores[m] = qk . h0T[:, m]
                    nc.tensor.matmul(misc[o:o + 64, 1:2],
                                     lhsT=kw2[o:o + 64, :],
                                     rhs=qe[o:o + 64, :],
                                     start=True, stop=True)
                qks = sm.tile([128, 1], F32, tag="qks")
                nc.vector.tensor_copy(qks[:], misc[:, 1:2])
                c1r = sm.tile([1, 2], F32, tag="c1r")
                nc.vector.tensor_copy(c1r[:], misc[0:1, 26:28])
                nc.tensor.matmul(misc[:, 2:4], lhsT=onesr[:], rhs=c1r[:],
                                 start=True, stop=True)
                # scores for both samples, interleaved like the stats tiles
                for j in range(NJ):
                    for s01 in range(2):
                        o = 64 * s01
                        nc.tensor.matmul(
                            misc[:, 4 + 2 * j + s01:5 + 2 * j + s01],
                            lhsT=h0[o:o + 64, 128 * j:128 * j + 128],
                            rhs=qks[o:o + 64, :], start=True, stop=True)
                mv3 = mean[:].rearrange("p (j t) -> p j t", t=2)
                tmp = sm.tile([128, 16], F32, tag="tmp")
                nc.vector.tensor_tensor(
                    tmp[:].rearrange("p (j t) -> p j t", t=2), mv3,
                    misc[:, 2:4][:, None].to_broadcast([128, NJ, 2]),
                    op=OP.mult)
                sc = sm.tile([128, 16], F32, tag="sc")
                nc.vector.tensor_sub(sc[:], misc[:, 4:20], tmp[:])
                nc.vector.tensor_mul(sc[:], sc[:], r8[:])
                exps = sm.tile([128, 16], F32, tag="exps")
                nc.scalar.activation(exps[:], sc[:], AF.Exp)
                nc.vector.tensor_scalar(exps[:, 14:16], exps[:, 14:16],
                                        lastm[:], None, op0=OP.mult)
                # Z per sample: reduce over chunks, then over partitions
                zs = sm.tile([128, 2], F32, tag="zs")
                nc.vector.tensor_reduce(
                    zs[:], exps[:].rearrange("p (j t) -> p t j", t=2),
                    mybir.AxisListType.X, OP.add)
                nc.tensor.matmul(misc[0:2, 28:29], lhsT=zs[:], rhs=ones128[:],
                                 start=True, stop=True)
                z2s = sm.tile([2, 1], F32, tag="z2s")
                nc.vector.tensor_copy(z2s[:], misc[0:2, 28:29])
                nc.tensor.matmul(misc[0:1, 29:31], lhsT=z2s[:],
                                 rhs=id4[0:2, 0:2], start=True, stop=True)
                rzr = sm.tile([1, 2], F32, tag="rzr")
                nc.vector.reciprocal(rzr[:], misc[0:1, 29:31])
                nc.tensor.matmul(misc[:, 32:34], lhsT=onesr[:], rhs=rzr[:],
                                 start=True, stop=True)
                # wr = exps/Z * rstd ; g = sum(wr * mu) per sample
                wr = sm.tile([128, 16], F32, tag="wr")
                nc.vector.tensor_tensor(
                    wr[:].rearrange("p (j t) -> p j t", t=2),
                    exps[:].rearrange("p (j t) -> p j t", t=2),
                    misc[:, 32:34][:, None].to_broadcast([128, NJ, 2]),
                    op=OP.mult)
                nc.vector.tensor_mul(wr[:], wr[:], rstd[:])
                gt = sm.tile([128, 16], F32, tag="gt")
                nc.vector.tensor_mul(gt[:], wr[:], mean[:])
                gs = sm.tile([128, 2], F32, tag="gs")
                nc.vector.tensor_reduce(
                    gs[:], gt[:].rearrange("p (j t) -> p t j", t=2),
                    mybir.AxisListType.X, OP.add)
                nc.tensor.matmul(misc[0:2, 34:35], lhsT=gs[:], rhs=ones128[:],
                                 start=True, stop=True)
                g2s = sm.tile([2, 1], F32, tag="g2s")
                nc.vector.tensor_copy(g2s[:], misc[0:2, 34:35])
                nc.tensor.matmul(misc[0:1, 35:37], lhsT=g2s[:],
                                 rhs=id4[0:2, 0:2], start=True, stop=True)
                grow = sm.tile([1, 2], F32, tag="grow")
                nc.vector.tensor_copy(grow[:], misc[0:1, 35:37])
                nc.tensor.matmul(misc[0:64, 37:39], lhsT=onesr[0:1, 0:64],
                                 rhs=grow[:], start=True, stop=True)
                # v0 + attention accumulate + out-proj, per sample
                for s01 in range(2):
                    s = 2 * pair + s01
                    o = 64 * s01
                    vps = ps.tile([128, 512], F32, tag="v0", bufs=2)
                    for j in range(NJ):
                        nc.tensor.matmul(vps[:, 64 * j:64 * j + 64],
                                         lhsT=h0[o:o + 64, 128 * j:128 * j + 128],
                                         rhs=vwt[o:o + 64, :],
                                         start=True, stop=True)
                    v0 = vp.tile([128, 512], F32, tag="v0sb")
                    nc.scalar.copy(v0[:], vps[:])
                    for j in range(NJ):
                        nc.tensor.matmul(
                            misc[0:64, 39 + s01:40 + s01],
                            lhsT=v0[:, 64 * j:64 * j + 64],
                            rhs=wr[:, 2 * j + s01:2 * j + s01 + 1],
                            start=(j == 0), stop=(j == NJ - 1))
                    oc = sm.tile([64, 1], F32, tag="oc")
                    nc.vector.tensor_mul(oc[:], svcol[:],
                                         misc[0:64, 37 + s01:38 + s01])
                    nc.vector.tensor_sub(oc[:], misc[0:64, 39 + s01:40 + s01],
                                         oc[:])
                    nc.tensor.matmul(misc[0:64, 41 + s01:42 + s01],
                                     lhsT=owt[:], rhs=oc[:],
                                     start=True, stop=True)
                    nc.vector.tensor_copy(ha[0:D, s:s + 1],
                                          misc[0:64, 41 + s01:42 + s01])

            # ---- batched tail over the 4 samples ----
            tl = ps.tile([128, 128], F32, tag="stats", bufs=2)
            eop = ps.tile([SPC, E * D], F32, tag="convA", bufs=2)
            nc.tensor.matmul(tl[0:SPC, 0:E], lhsT=ha[0:D, :], rhs=rwt[:],
                             start=True, stop=True)
            el = sm.tile([SPC, E], F32, tag="el")
            nc.scalar.activation(el[:], tl[0:SPC, 0:E], AF.Exp)
            zr = sm.tile([SPC, 1], F32, tag="zr")
            nc.vector.tensor_reduce(zr[:], el[:], mybir.AxisListType.X, OP.add)
            rr = sm.tile([SPC, 1], F32, tag="rr")
            nc.vector.reciprocal(rr[:], zr[:])
            rw = sm.tile([SPC, E], F32, tag="rw")
            nc.vector.tensor_scalar(rw[:], el[:], rr[:], None, op0=OP.mult)
            m8 = sm.tile([SPC, 8], F32, tag="m8")
            nc.vector.max(m8[:], rw[:])
            msk = sm.tile([SPC, E], F32, tag="msk")
            nc.vector.tensor_scalar(msk[:], rw[:], m8[:, TOPK - 1:TOPK], None,
                                    op0=OP.is_ge)
            w4 = sm.tile([SPC, E], F32, tag="w4")
            nc.vector.tensor_mul(w4[:], rw[:], msk[:])
            # expert outputs (dense) and weighted sum over selected experts
            nc.tensor.matmul(eop[:], lhsT=ha[:], rhs=wexp[:],
                             start=True, stop=True)
            prod = sm.tile([SPC, E * D], F32, tag="prod")
            nc.vector.tensor_tensor(
                prod[:].rearrange("p (e d) -> p e d", e=E), eop[:].rearrange("p (e d) -> p e d", e=E),
                w4[:].to_broadcast([SPC, E, D]), op=OP.mult)
            s1 = sm.tile([SPC, 256], F32, tag="s1")
            nc.vector.tensor_add(s1[:], prod[:, 0:256], prod[:, 256:512])
            s2 = sm.tile([SPC, 128], F32, tag="s2")
            nc.vector.tensor_add(s2[:], s1[:, 0:128], s1[:, 128:256])
            moe4 = sm.tile([SPC, D], F32, tag="moe4")
            nc.vector.tensor_add(moe4[:], s2[:, 0:64], s2[:, 64:128])
            # transpose to (64, 4), project through moe_out_w
            nc.tensor.transpose(tl[0:D, 8:8 + SPC], moe4[:], id4[:])
            moet = sm.tile([D, SPC], F32, tag="moet")
            nc.vector.tensor_copy(moet[:], tl[0:D, 8:8 + SPC])
            nc.tensor.matmul(tl[0:D, 16:16 + SPC], lhsT=mowt[:], rhs=moet[:],
                             start=True, stop=True)
            hm = sm.tile([D, SPC], F32, tag="hm")
            nc.vector.tensor_copy(hm[:], tl[0:D, 16:16 + SPC])
            # LN2
            nc.tensor.matmul(tl[0:1, 24:24 + SPC], lhsT=ones128[0:D, :],
                             rhs=hm[:], start=True, stop=True)
            mu2 = sm.tile([1, SPC], F32, tag="mu2")
            nc.scalar.activation(mu2[:], tl[0:1, 24:24 + SPC], AF.Copy,
                                 scale=1.0 / D)
            nc.tensor.matmul(tl[0:D, 28:28 + SPC], lhsT=onesr[0:1, 0:D],
                             rhs=mu2[:], start=True, stop=True)
            hc = sm.tile([D, SPC], F32, tag="hc")
            nc.vector.tensor_sub(hc[:], hm[:], tl[0:D, 28:28 + SPC])
            sq2 = sm.tile([D, SPC], F32, tag="sq2")
            nc.scalar.activation(sq2[:], hc[:], AF.Square)
            nc.tensor.matmul(tl[0:1, 24 + SPC:24 + 2 * SPC],
                             lhsT=ones128[0:D, :], rhs=sq2[:],
                             start=True, stop=True)
            var2 = sm.tile([1, SPC], F32, tag="var2")
            nc.scalar.activation(var2[:], tl[0:1, 24 + SPC:24 + 2 * SPC],
                                 AF.Copy, scale=1.0 / D)
            std2 = sm.tile([1, SPC], F32, tag="std2")
            nc.scalar.activation(std2[:], var2[:], AF.Sqrt, bias=epsb[0:1, :])
            rstd2 = sm.tile([1, SPC], F32, tag="rstd2")
            nc.vector.reciprocal(rstd2[:], std2[:])
            nc.tensor.matmul(tl[0:SPC, 40:41], lhsT=rstd2[:],
                             rhs=onesr[0:1, 0:1], start=True, stop=True)
            rsc = sm.tile([SPC, 1], F32, tag="rsc")
            nc.vector.tensor_copy(rsc[:], tl[0:SPC, 40:41])
            # final projection, scaled by rstd2 per row on eviction
            nc.tensor.matmul(tl[0:SPC, 32:32 + PRED], lhsT=hc[:], rhs=outwt[:],
                             start=True, stop=True)
            outp = sm.tile([SPC, PRED], F32, tag="outp")
            nc.scalar.activation(outp[:], tl[0:SPC, 32:32 + PRED], AF.Copy,
                                 scale=rsc[:])
            nc.sync.dma_start(Yout.ap(), outp[:])

    nc.compile()
    return nc


_NC_CACHE = {}


def _get_nc():
    if "nc" not in _NC_CACHE:
        _NC_CACHE["nc"] = build_nc()
    return _NC_CACHE["nc"]


def _prep_in_maps(inputs):
    f32 = np.float32
    X = np.ascontiguousarray(inputs["X"], f32)
    conv_w = np.asarray(inputs["conv_w"], f32)
    conv_b = np.asarray(inputs["conv_b"], f32)
    qw, kw, vw, ow = (np.asarray(inputs[k], f32) for k in ("qw", "kw", "vw", "ow"))
    expert_w = np.asarray(inputs["expert_w"], f32)
    expert_b = np.asarray(inputs["expert_b"], f32)
    router_w = np.asarray(inputs["router_w"], f32)
    moe_out_w = np.asarray(inputs["moe_out_w"], f32)
    out_w = np.asarray(inputs["out_w"], f32)

    np_x = mybir.dt.np(DT_X)
    Wc = np.ascontiguousarray(conv_w.transpose(2, 1, 0).reshape(C * P, D)).astype(np_x)
    pebT = (_pos_encoding_np(N, D) + conv_b[None, :]).T.astype(f32)  # (64, N)
    PEBT2 = np.ascontiguousarray(np.concatenate([pebT, pebT], axis=0))
    dbl = lambda a: np.ascontiguousarray(np.concatenate([a, a], axis=0), dtype=f32)
    QwT2 = dbl(qw.T)
    Kw2 = dbl(kw)
    VwT2 = dbl(vw.T)
    SqCol2 = dbl(qw.sum(1)[:, None])
    SkCol2 = dbl(kw.sum(1)[:, None])
    SvCol = np.ascontiguousarray(vw.sum(1)[:, None], dtype=f32)
    SelAB = np.zeros((128, 2), f32)
    SelAB[0:64, 0] = 1.0
    SelAB[64:128, 1] = 1.0
    OnesRow = np.ones((1, 128), f32)
    Ones128 = np.ones((128, 1), f32)
    OneHot = np.zeros((128, 1), f32)
    OneHot[126, 0] = 1.0
    LastMask = np.ones((128, 1), f32)
    LastMask[127, 0] = 0.0
    WexpE = np.concatenate(
        [expert_w.transpose(2, 0, 1).reshape(D, E * D),
         expert_b.reshape(1, E * D)], axis=0).astype(f32)
    common = dict(
        Wc=Wc, PEBT2=PEBT2, QwT2=QwT2, Kw2=Kw2, VwT2=VwT2,
        OwT=np.ascontiguousarray(ow.T), SqCol2=SqCol2, SkCol2=SkCol2,
        SvCol=SvCol, SelAB=SelAB, OnesRow=OnesRow, Ones128=Ones128,
        OneHot=OneHot, LastMask=LastMask,
        RwT=np.ascontiguousarray(router_w.T),
        WexpE=np.ascontiguousarray(WexpE),
        MowT=np.ascontiguousarray(moe_out_w.T),
        OutWT=np.ascontiguousarray(out_w.T),
        Id4=np.eye(SPC, dtype=f32),
    )
    common = {k: np.ascontiguousarray(v, dtype=f32) for k, v in common.items()}
    common["Wc"] = Wc
    in_maps = []
    for c in range(NCORES):
        m = dict(common)
        m["Xs"] = np.ascontiguousarray(X[c * SPC:(c + 1) * SPC]).astype(np_x)
        in_maps.append(m)
    return in_maps


def kernel(**inputs) -> np.ndarray:
    nc = _get_nc()
    in_maps = _prep_in_maps(inputs)
    res = run_bass_kernel_spmd(nc, in_maps, core_ids=list(range(NCORES)))
    out = np.concatenate([res.results[c]["Yout"] for c in range(NCORES)], axis=0)
    return out.astype(np.float32)



# revision 5
# speedup vs baseline: 1.8311x; 1.8311x over previous
"""Trainium2 Bass kernel for nn_Decoder_22703197127089 (moe_routing).

Only each sample's LAST token survives to the output (h[:, -1, :] after the
MoE block), so attention rows 0..N-2 and the dense MoE block are dead code.
Per sample: conv patch-embed (1023 tokens) -> LN1 -> one attention row ->
out-proj -> MoE for 1 token -> LN2 -> final linear (96).

Sharding: data-parallel over batch B=32 across 8 cores (4 samples/core),
no collectives; host gathers per-core (4, 96) outputs.

Conv: X de-interleaved on host (T[c,m]=X[c,2m], T[64+c,m]=X[c,2m+1]) so a
K=128 chunk j covers patch offsets {2j,2j+1} x 64 channels at ONE column
6m+j.  The 24-tap patch splits into two 12-tap halves packed as matmul
output rows [0:64]=A / [64:128]=B, giving h[:, n] = A[:, n] + B[:, n+1]:
6 chunks x 1024 columns/sample.  PE bias + positional encoding is injected
into the same PSUM group via a selector matmul; eviction = one Act copy
(shifted B half) + one DVE add.

Phase B (LN1 stats -> attention row -> out-proj) is BATCHED across all 4
samples: every element-wise stage runs once on [128, 32] tiles laid out
col = 4j + 2q + s01 (j = token chunk, q = pair, s01 = sample in pair).
Last-token mu/rstd are extracted from the batched stats via onehot
matmuls.  All rsqrt are DVE-only (bit hack + 2 Newton steps) so the Act
engine stays on one table set (exp/copy/square -> single LoadActFuncSet).
Softmax is unnormalized; 1/Z and the LN1 /8 fold into one reciprocal row.
"""

import numpy as np

import concourse.bass as bass
import concourse.mybir as mybir
import concourse.tile as tile
from concourse import bacc
from concourse.bass_utils import run_bass_kernel_spmd

F32 = mybir.dt.float32
BF16 = mybir.dt.bfloat16
I32 = mybir.dt.int32
AF = mybir.ActivationFunctionType
OP = mybir.AluOpType
AX = mybir.AxisListType

B, C, L = 32, 64, 12288
D = 64
E = 8
TOPK = 4
P, S = 24, 12
PRED = 96
N = (L - P) // S + 1  # 1023
NCORES = 8
SPC = B // NCORES
NPAIR = SPC // 2
M6 = L // 2
NJ = 8
EPS = 1e-5

# ---- CB16 (bf16 const block) columns ----
C_W6 = 0
C_W6S = 768
C_SELA = 1536
C_SELB = 1664
C_PEB = 1792
C_QWT = 2816
C_KW = 2880
C_VWT = 2944
C_OWT = 3008
C_SELAB = 3072
C_ONEC = 3074
C_SKC = 3075
C_RWT = 3076
C_WEXP = 3084
C_MOWT = 3596
C_OUTWT = 3660
C_OWSN = 3756
C_ID4 = 3852
NC16 = 3856

# ---- CB32 (f32 const block) columns ----
Z_SQC = 0         # qw.sum(1) col (rows 0:64)
Z_SVC = 1         # vw.sum(1) col (rows 0:64)
Z_LASTM = 2       # ones, 0 at partition 127
Z_OH = 3          # onehot at partition 126
Z_ONESR = 5       # ones row [1, 128]
Z_IDZG8 = 133     # diag(1/8 x4, 1 x4) rows 0:8
Z_ID4 = 141       # I4 rows 0:4
Z_ONEC = 145      # ones col
NC32 = 146

# ---- shared phase-B PSUM tile (pb, [128, 128]) columns ----
PB_SUM = 0    # 32: per-token sums   (col = 4j + 2q + s01); scores reuse
PB_SSQ = 32   # 32: per-token sumsq
PB_Q0 = 64    # 4:  q0 (rows 0:64)
PB_XR = 68    # 8:  extract row [1,8] = [mu_l x4 | r8_l x4]
PB_BC = 76    # 8:  [64,8] broadcast of extract row
PB_QK = 84    # 4:  qk (rows 64*s01)
PB_C1 = 88    # 4:  c1 row [1,4]
PB_CB = 92    # 4:  c1 broadcast [128,4]
PB_ZG = 96    # 1:  [8,1] Z/G column sums
PB_ZR = 97    # 8:  [1,8] = [Z/8 x4 | G8 x4]
PB_ZB = 105   # 8:  [64,8] broadcast [8/Z x4 | G8 x4]
PB_AT = 113   # 4:  attention accumulators [64,4]
PB_HA = 117   # 4:  out-proj results [64,4]


def _pos_encoding_np(n, d):
    pos = np.arange(n, dtype=np.float32)[:, None]
    div = np.exp(np.arange(0, d, 2, dtype=np.float32)
                 * (np.float32(-np.log(np.float32(10000.0))) / np.float32(d)))
    pe = np.zeros((n, d), np.float32)
    pe[:, 0::2] = np.sin(pos * div)
    pe[:, 1::2] = np.cos(pos * div)
    return pe


def build_nc():
    nc = bacc.Bacc("TRN2", target_bir_lowering=False, debug=False,
                   num_devices=NCORES)

    Xd = nc.dram_tensor("Xd", [SPC, 128, M6], BF16, kind="ExternalInput")
    CB16 = nc.dram_tensor("CB16", [128, NC16], BF16, kind="ExternalInput")
    CB32 = nc.dram_tensor("CB32", [128, NC32], F32, kind="ExternalInput")
    Yout = nc.dram_tensor("Yout", [SPC, PRED], F32, kind="ExternalOutput")

    with tile.TileContext(nc) as tc:
        with (
            tc.tile_pool(name="const", bufs=1) as pc,
            tc.tile_pool(name="xp", bufs=1) as xp,
            tc.tile_pool(name="hp", bufs=1) as hp,
            tc.tile_pool(name="sp", bufs=1) as sp,
            tc.tile_pool(name="ps", bufs=1, space="PSUM") as ps,
        ):
            cb16 = pc.tile([128, NC16], BF16, tag="cb16", name="cb16")
            nc.sync.dma_start(cb16[:], CB16.ap())
            cb32 = pc.tile([128, NC32], F32, tag="cb32", name="cb32")
            nc.sync.dma_start(cb32[:], CB32.ap())

            ha = pc.tile([D + 1, SPC], BF16, tag="ha", name="ha")
            nc.vector.memset(ha[D:D + 1, :], 1.0)
            qksb = pc.tile([128, SPC], BF16, tag="qksb", name="qksb")
            nc.vector.memset(qksb[:], 0.0)
            magic = pc.tile([128, 32], I32, tag="magic", name="magic")
            nc.vector.memset(magic[:], 0x5F3759DF)

            def rsqrt_dve(dst, x, pp, ff, tagb, final_scale=1.0, iters=2):
                # dst = final_scale / sqrt(x); Newton on DVE only.
                ish = sp.tile([pp, ff], I32, tag=tagb + "_i", name="ish")
                nc.vector.tensor_scalar(ish[:], x.bitcast(I32), 1, None,
                                        op0=OP.logical_shift_right)
                y = sp.tile([pp, ff], F32, tag=tagb + "_y", name="y")
                nc.vector.tensor_sub(y[:].bitcast(I32), magic[0:pp, 0:ff],
                                     ish[:])
                for it in range(iters):
                    last = it == iters - 1
                    t1 = sp.tile([pp, ff], F32, tag=tagb + "_a", name="t1")
                    nc.vector.tensor_mul(t1[:], y[:], y[:])
                    t2 = sp.tile([pp, ff], F32, tag=tagb + "_b", name="t2")
                    nc.vector.tensor_mul(t2[:], t1[:], x)
                    t3 = sp.tile([pp, ff], F32, tag=tagb + "_c", name="t3")
                    cs = final_scale if last else 1.0
                    nc.vector.tensor_scalar(t3[:], t2[:], -0.5 * cs, 1.5 * cs,
                                            op0=OP.mult, op1=OP.add)
                    dd = dst if last else sp.tile([pp, ff], F32,
                                                  tag=tagb + "_d", name="dd")
                    nc.vector.tensor_mul(dd[:], y[:], t3[:])
                    y = dd

            # phase-B shared PSUM tile lives the whole kernel
            pb = ps.tile([128, 128], F32, tag="pb", name="pb", bufs=2)

            xd = []
            for s in range(SPC):
                t = xp.tile([128, M6], BF16, tag=f"xd{s}", name=f"xd{s}")
                nc.sync.dma_start(t[:, 0:M6 // 2], Xd.ap()[s][:, 0:M6 // 2])
                nc.sync.dma_start(t[:, M6 // 2:M6], Xd.ap()[s][:, M6 // 2:M6])
                xd.append(t)

            h0 = {}

            def conv_sample(s):
                pair, odd = divmod(s, 2)
                o = 64 * odd
                w0 = C_W6S if odd else C_W6
                selc = C_SELB if odd else C_SELA
                if not odd:
                    h0[pair] = hp.tile([128, 1024], BF16, tag=f"h0_{pair}",
                                       name=f"h0_{pair}")
                    nc.vector.memset(h0[pair][:, 1023:1024], 0.0)
                cps = ps.tile([128, 1024], F32, tag="conv", name=f"cps{s}",
                              bufs=2)
                xv = xd[s][:].rearrange("p (n k) -> p n k", k=6)
                for mc in range(2):
                    cc = slice(512 * mc, 512 * mc + 512)
                    nc.tensor.matmul(
                        cps[:, cc], lhsT=cb16[:, selc:selc + 128],
                        rhs=cb16[:, C_PEB + 512 * mc:C_PEB + 512 * mc + 512],
                        start=True, stop=False)
                for j in range(6):
                    wj = cb16[:, w0 + 128 * j:w0 + 128 * j + 128]
                    for mc in range(2):
                        cc = slice(512 * mc, 512 * mc + 512)
                        nc.tensor.matmul(
                            cps[:, cc], lhsT=wj,
                            rhs=xv[:, 512 * mc:512 * mc + 512, j],
                            start=False, stop=(j == 5))
                bo = 64 - o
                tmp = hp.tile([128, 1023], F32, tag="tmpB", name=f"tmpB{s}",
                              bufs=2)
                nc.scalar.copy(tmp[o:o + 64, :], cps[bo:bo + 64, 1:1024])
                nc.vector.tensor_add(h0[pair][o:o + 64, 0:1023],
                                     cps[o:o + 64, 0:1023], tmp[o:o + 64, :])

            v0sb = {}

            def phase_b_early(pair):
                h0p = h0[pair]
                sqt = sp.tile([128, 1024], BF16, tag=f"sq_{pair}",
                              name=f"sq_{pair}")
                nc.scalar.activation(sqt[:], h0p[:], AF.Square)
                for j in range(NJ):
                    c = 4 * j + 2 * pair
                    nc.tensor.matmul(pb[:, PB_SUM + c:PB_SUM + c + 2],
                                     lhsT=h0p[:, 128 * j:128 * j + 128],
                                     rhs=cb16[:, C_SELAB:C_SELAB + 2],
                                     start=True, stop=True)
                for j in range(NJ):
                    c = 4 * j + 2 * pair
                    nc.tensor.matmul(pb[:, PB_SSQ + c:PB_SSQ + c + 2],
                                     lhsT=sqt[:, 128 * j:128 * j + 128],
                                     rhs=cb16[:, C_SELAB:C_SELAB + 2],
                                     start=True, stop=True)
                for s01 in range(2):
                    o = 64 * s01
                    s = 2 * pair + s01
                    vps = ps.tile([128, 512], F32, tag="vps",
                                  name=f"vps{s}", bufs=2)
                    for j in range(NJ):
                        nc.tensor.matmul(
                            vps[:, 64 * j:64 * j + 64],
                            lhsT=h0p[o:o + 64, 128 * j:128 * j + 128],
                            rhs=cb16[o:o + 64, C_VWT:C_VWT + 64],
                            start=True, stop=True)
                    v0sb[s] = sp.tile([128, 512], BF16, tag=f"v0_{s}",
                                      name=f"v0_{s}")
                    nc.scalar.copy(v0sb[s][:], vps[:])
                    # q0 = qw @ h0_last
                    nc.tensor.matmul(pb[0:64, PB_Q0 + s:PB_Q0 + s + 1],
                                     lhsT=cb16[o:o + 64, C_QWT:C_QWT + 64],
                                     rhs=h0p[o:o + 64, N - 1:N],
                                     start=True, stop=True)

            def batched_chain():
                # ---- LN1 stats for all 4 samples ----
                mex = sp.tile([128, 32], F32, tag="mex", name="mex")
                nc.vector.tensor_scalar_mul(mex[:], pb[:, PB_SUM:PB_SUM + 32],
                                            1.0 / D)
                ex2 = sp.tile([128, 32], F32, tag="ex2", name="ex2")
                nc.vector.tensor_scalar(ex2[:], pb[:, PB_SSQ:PB_SSQ + 32],
                                        1.0 / D, EPS, op0=OP.mult, op1=OP.add)
                msq = sp.tile([128, 32], F32, tag="msq", name="msq")
                nc.vector.tensor_mul(msq[:], mex[:], mex[:])
                vart = sp.tile([128, 32], F32, tag="vart", name="vart")
                nc.vector.tensor_sub(vart[:], ex2[:], msq[:])
                r8 = sp.tile([128, 32], F32, tag="r8", name="r8")
                rsqrt_dve(r8, vart[:], 128, 32, "rsv", final_scale=0.125)
                rm8 = sp.tile([128, 32], F32, tag="rm8", name="rm8")
                nc.vector.tensor_mul(rm8[:], r8[:], mex[:])
                # ---- last-token mu/rstd extraction + q_eff ----
                for q in range(NPAIR):
                    for s01 in range(2):
                        c = 28 + 2 * q + s01
                        d0 = PB_XR + 2 * q + s01
                        nc.tensor.matmul(pb[0:1, d0:d0 + 1],
                                         lhsT=mex[:, c:c + 1],
                                         rhs=cb32[:, Z_OH:Z_OH + 1],
                                         start=True, stop=True)
                        nc.tensor.matmul(pb[0:1, d0 + 4:d0 + 5],
                                         lhsT=r8[:, c:c + 1],
                                         rhs=cb32[:, Z_OH:Z_OH + 1],
                                         start=True, stop=True)
                xrow = sp.tile([1, 8], F32, tag="xrow", name="xrow")
                nc.vector.tensor_copy(xrow[:], pb[0:1, PB_XR:PB_XR + 8])
                nc.tensor.matmul(pb[0:64, PB_BC:PB_BC + 8],
                                 lhsT=cb32[0:1, Z_ONESR:Z_ONESR + 64],
                                 rhs=xrow[:], start=True, stop=True)
                t1 = sp.tile([64, SPC], F32, tag="t1q", name="t1q")
                nc.vector.tensor_scalar(t1[:], pb[0:64, PB_BC:PB_BC + 4],
                                        cb32[0:64, Z_SQC:Z_SQC + 1], None,
                                        op0=OP.mult)
                t2 = sp.tile([64, SPC], F32, tag="t2q", name="t2q")
                nc.vector.tensor_sub(t2[:], pb[0:64, PB_Q0:PB_Q0 + 4], t1[:])
                qe = sp.tile([64, SPC], BF16, tag="qe", name="qe")
                nc.vector.scalar_tensor_tensor(
                    qe[:], t2[:], 8.0, pb[0:64, PB_BC + 4:PB_BC + 8],
                    op0=OP.mult, op1=OP.mult)
                # qk = kw.T @ q_eff; c1 = qk . 1
                for c in range(SPC):
                    s01 = c % 2
                    o = 64 * s01
                    nc.tensor.matmul(pb[o:o + 64, PB_QK + c:PB_QK + c + 1],
                                     lhsT=cb16[0:64, C_KW:C_KW + 64],
                                     rhs=qe[:, c:c + 1],
                                     start=True, stop=True)
                    nc.tensor.matmul(pb[0:1, PB_C1 + c:PB_C1 + c + 1],
                                     lhsT=qe[:, c:c + 1],
                                     rhs=cb16[0:64, C_SKC:C_SKC + 1],
                                     start=True, stop=True)
                nc.vector.tensor_copy(qksb[0:64, 0:4:2],
                                      pb[0:64, PB_QK:PB_QK + 4:2])
                nc.vector.tensor_copy(qksb[64:128, 1:4:2],
                                      pb[64:128, PB_QK + 1:PB_QK + 4:2])
                c1r = sp.tile([1, SPC], F32, tag="c1r", name="c1r")
                nc.vector.tensor_copy(c1r[:], pb[0:1, PB_C1:PB_C1 + 4])
                nc.tensor.matmul(pb[:, PB_CB:PB_CB + 4],
                                 lhsT=cb32[0:1, Z_ONESR:Z_ONESR + 128],
                                 rhs=c1r[:], start=True, stop=True)
                # ---- scores (reuse pb cols 0:32 after stats consumed) ----
                for q in range(NPAIR):
                    for j in range(NJ):
                        c = 4 * j + 2 * q
                        nc.tensor.matmul(pb[:, c:c + 2],
                                         lhsT=h0[q][:, 128 * j:128 * j + 128],
                                         rhs=qksb[:, 2 * q:2 * q + 2],
                                         start=True, stop=True)
                tt = sp.tile([128, 32], F32, tag="tt", name="tt")
                nc.vector.tensor_tensor(
                    tt[:].rearrange("p (j v) -> p j v", v=4),
                    mex[:].rearrange("p (j v) -> p j v", v=4),
                    pb[:, PB_CB:PB_CB + 4][:, None].to_broadcast([128, NJ, 4]),
                    op=OP.mult)
                sc = sp.tile([128, 32], F32, tag="sc", name="sc")
                nc.vector.tensor_sub(sc[:], pb[:, 0:32], tt[:])
                nc.vector.tensor_mul(sc[:], sc[:], r8[:])
                ex = sp.tile([128, 32], F32, tag="ex", name="ex")
                nc.scalar.activation(ex[:], sc[:], AF.Exp)
                nc.vector.tensor_scalar(ex[:, 28:32], ex[:, 28:32],
                                        cb32[:, Z_LASTM:Z_LASTM + 1], None,
                                        op0=OP.mult)
                wr = sp.tile([128, 32], BF16, tag="wr", name="wr")
                nc.vector.tensor_mul(wr[:], ex[:], r8[:])
                # attention accumulation (gated on wr only)
                for q in range(NPAIR):
                    for s01 in range(2):
                        s = 2 * q + s01
                        c = 2 * q + s01
                        for j in range(NJ):
                            nc.tensor.matmul(
                                pb[0:64, PB_AT + c:PB_AT + c + 1],
                                lhsT=v0sb[s][:, 64 * j:64 * j + 64],
                                rhs=wr[:, 4 * j + c:4 * j + c + 1],
                                start=(j == 0), stop=(j == NJ - 1))
                # Z / G8 path
                zg = sp.tile([128, 8], F32, tag="zg", name="zg")
                nc.vector.tensor_reduce(
                    zg[:, 0:4], ex[:].rearrange("p (j v) -> p v j", v=4),
                    AX.X, OP.add)
                gt = sp.tile([128, 32], F32, tag="gt", name="gt")
                nc.vector.tensor_mul(gt[:], ex[:], rm8[:])
                nc.vector.tensor_reduce(
                    zg[:, 4:8], gt[:].rearrange("p (j v) -> p v j", v=4),
                    AX.X, OP.add)
                nc.tensor.matmul(pb[0:8, PB_ZG:PB_ZG + 1], lhsT=zg[:],
                                 rhs=cb32[:, Z_ONEC:Z_ONEC + 1],
                                 start=True, stop=True)
                zcol = sp.tile([8, 1], F32, tag="zcol", name="zcol")
                nc.vector.tensor_copy(zcol[:], pb[0:8, PB_ZG:PB_ZG + 1])
                nc.tensor.matmul(pb[0:1, PB_ZR:PB_ZR + 8], lhsT=zcol[:],
                                 rhs=cb32[0:8, Z_IDZG8:Z_IDZG8 + 8],
                                 start=True, stop=True)
                rr8 = sp.tile([1, 8], F32, tag="rr8", name="rr8")
                nc.vector.reciprocal(rr8[0:1, 0:4], pb[0:1, PB_ZR:PB_ZR + 4])
                nc.vector.tensor_copy(rr8[0:1, 4:8],
                                      pb[0:1, PB_ZR + 4:PB_ZR + 8])
                nc.tensor.matmul(pb[0:64, PB_ZB:PB_ZB + 8],
                                 lhsT=cb32[0:1, Z_ONESR:Z_ONESR + 64],
                                 rhs=rr8[:], start=True, stop=True)
                # att = (att_u - G8*svcol) * (8/Z), then out-proj
                u1 = sp.tile([64, SPC], F32, tag="u1", name="u1")
                nc.vector.tensor_scalar(u1[:], pb[0:64, PB_ZB + 4:PB_ZB + 8],
                                        cb32[0:64, Z_SVC:Z_SVC + 1], None,
                                        op0=OP.mult)
                u2 = sp.tile([64, SPC], F32, tag="u2", name="u2")
                nc.vector.tensor_sub(u2[:], pb[0:64, PB_AT:PB_AT + 4], u1[:])
                oc = sp.tile([64, SPC], BF16, tag="oc", name="oc")
                nc.vector.tensor_mul(oc[:], u2[:], pb[0:64, PB_ZB:PB_ZB + 4])
                nc.tensor.matmul(pb[0:64, PB_HA:PB_HA + 4],
                                 lhsT=cb16[0:64, C_OWT:C_OWT + 64],
                                 rhs=oc[:], start=True, stop=True)
                nc.vector.tensor_copy(ha[0:64, :], pb[0:64, PB_HA:PB_HA + 4])

            def tail():
                tl = ps.tile([128, 128], F32, tag="pb", name="tail", bufs=2)
                ep = ps.tile([128, 512], F32, tag="vps", name="eop", bufs=2)
                nc.tensor.matmul(tl[0:SPC, 0:E], lhsT=ha[0:64, :],
                                 rhs=cb16[0:64, C_RWT:C_RWT + E],
                                 start=True, stop=True)
                nc.tensor.matmul(ep[0:SPC, :], lhsT=ha[:],
                                 rhs=cb16[0:D + 1, C_WEXP:C_WEXP + E * D],
                                 start=True, stop=True)
                el = sp.tile([SPC, E], F32, tag="el", name="el")
                nc.scalar.activation(el[:], tl[0:SPC, 0:E], AF.Exp)
                zr = sp.tile([SPC, 1], F32, tag="zr", name="zr")
                nc.vector.tensor_reduce(zr[:], el[:], AX.X, OP.add)
                rr = sp.tile([SPC, 1], F32, tag="rrt", name="rrt")
                nc.vector.reciprocal(rr[:], zr[:])
                m8 = sp.tile([SPC, 8], F32, tag="m8", name="m8")
                nc.vector.max(m8[:], el[:])
                w4n = sp.tile([SPC, E], F32, tag="w4n", name="w4n")
                nc.vector.scalar_tensor_tensor(w4n[:], el[:],
                                               m8[:, TOPK - 1:TOPK], el[:],
                                               op0=OP.is_ge, op1=OP.mult)
                w4 = sp.tile([SPC, E], F32, tag="w4", name="w4")
                nc.vector.tensor_scalar(w4[:], w4n[:], rr[:], None,
                                        op0=OP.mult)
                prod = sp.tile([SPC, E * D], F32, tag="prod", name="prod")
                nc.vector.tensor_tensor(
                    prod[:].rearrange("p (e d) -> p e d", e=E),
                    ep[0:SPC, :].rearrange("p (e d) -> p e d", e=E),
                    w4[:].to_broadcast([SPC, E, D]), op=OP.mult)
                moe4 = sp.tile([SPC, D], F32, tag="moe4", name="moe4")
                nc.vector.tensor_reduce(
                    moe4[:], prod[:].rearrange("p (e d) -> p d e", e=E),
                    AX.X, OP.add)
                nc.tensor.transpose(tl[0:64, 14:18], moe4[:],
                                    cb32[0:4, Z_ID4:Z_ID4 + 4])
                moet = sp.tile([64, SPC], BF16, tag="moet", name="moet")
                nc.vector.tensor_copy(moet[:], tl[0:64, 14:18])
                nc.tensor.matmul(tl[0:64, 18:22],
                                 lhsT=cb16[0:64, C_MOWT:C_MOWT + 64],
                                 rhs=moet[:], start=True, stop=True)
                hmsb = sp.tile([64, SPC], BF16, tag="hmsb", name="hmsb")
                nc.vector.tensor_copy(hmsb[:], tl[0:64, 18:22])
                sq2 = sp.tile([64, SPC], F32, tag="sq2", name="sq2")
                nc.scalar.activation(sq2[:], hmsb[:], AF.Square)
                nc.tensor.matmul(tl[0:SPC, 8:9], lhsT=hmsb[:],
                                 rhs=cb16[0:64, C_ONEC:C_ONEC + 1],
                                 start=True, stop=True)
                nc.tensor.matmul(tl[0:SPC, 9:10], lhsT=sq2[:],
                                 rhs=cb32[0:64, Z_ONEC:Z_ONEC + 1],
                                 start=True, stop=True)
                mu2 = sp.tile([SPC, 1], F32, tag="mu2", name="mu2")
                nc.vector.tensor_scalar_mul(mu2[:], tl[0:SPC, 8:9], 1.0 / D)
                e22 = sp.tile([SPC, 1], F32, tag="e22", name="e22")
                nc.vector.tensor_scalar(e22[:], tl[0:SPC, 9:10], 1.0 / D,
                                        EPS, op0=OP.mult, op1=OP.add)
                mq = sp.tile([SPC, 1], F32, tag="mq", name="mq")
                nc.vector.tensor_mul(mq[:], mu2[:], mu2[:])
                var2 = sp.tile([SPC, 1], F32, tag="var2", name="var2")
                nc.vector.tensor_sub(var2[:], e22[:], mq[:])
                rstd2 = sp.tile([SPC, 1], F32, tag="rstd2", name="rstd2")
                rsqrt_dve(rstd2, var2[:], SPC, 1, "rs2")
                mu2b = sp.tile([SPC, 1], BF16, tag="mu2b", name="mu2b")
                nc.vector.tensor_copy(mu2b[:], mu2[:])
                nc.tensor.matmul(tl[0:1, 10:14], lhsT=mu2b[:],
                                 rhs=cb16[0:4, C_ID4:C_ID4 + 4],
                                 start=True, stop=True)
                mu2r = sp.tile([1, SPC], BF16, tag="mu2r", name="mu2r")
                nc.vector.tensor_copy(mu2r[:], tl[0:1, 10:14])
                nc.tensor.matmul(tl[0:SPC, 22:22 + PRED], lhsT=hmsb[:],
                                 rhs=cb16[0:64, C_OUTWT:C_OUTWT + PRED],
                                 start=True, stop=False)
                nc.tensor.matmul(tl[0:SPC, 22:22 + PRED], lhsT=mu2r[:],
                                 rhs=cb16[0:1, C_OWSN:C_OWSN + PRED],
                                 start=False, stop=True)
                outsb = sp.tile([SPC, PRED], F32, tag="outsb", name="outsb")
                nc.scalar.activation(outsb[:], tl[0:SPC, 22:22 + PRED],
                                     AF.Copy, scale=rstd2[:])
                nc.sync.dma_start(Yout.ap(), outsb[:])

            conv_sample(0)
            conv_sample(1)
            phase_b_early(0)
            conv_sample(2)
            conv_sample(3)
            phase_b_early(1)
            batched_chain()
            tail()

    nc.compile()
    return nc


_NC_CACHE = {}


def _get_nc():
    if "nc" not in _NC_CACHE:
        _NC_CACHE["nc"] = build_nc()
    return _NC_CACHE["nc"]


def _prep_in_maps(inputs):
    f32 = np.float32
    bf16 = mybir.dt.np(BF16)
    X = np.asarray(inputs["X"], f32)
    conv_w = np.asarray(inputs["conv_w"], f32)
    conv_b = np.asarray(inputs["conv_b"], f32)
    qw, kw, vw, ow = (np.asarray(inputs[k], f32)
                      for k in ("qw", "kw", "vw", "ow"))
    expert_w = np.asarray(inputs["expert_w"], f32)
    expert_b = np.asarray(inputs["expert_b"], f32)
    router_w = np.asarray(inputs["router_w"], f32)
    moe_out_w = np.asarray(inputs["moe_out_w"], f32)
    out_w = np.asarray(inputs["out_w"], f32)

    cb16 = np.zeros((128, NC16), f32)
    wt = conv_w.transpose(1, 2, 0)  # (C, P, D)
    for j in range(6):
        blk = np.zeros((128, 128), f32)
        blk[0:64, 0:64] = wt[:, 2 * j, :]
        blk[64:128, 0:64] = wt[:, 2 * j + 1, :]
        blk[0:64, 64:128] = wt[:, 12 + 2 * j, :]
        blk[64:128, 64:128] = wt[:, 13 + 2 * j, :]
        cb16[:, C_W6 + 128 * j:C_W6 + 128 * j + 128] = blk
        swp = np.concatenate([blk[:, 64:128], blk[:, 0:64]], axis=1)
        cb16[:, C_W6S + 128 * j:C_W6S + 128 * j + 128] = swp
    sela = np.zeros((128, 128), f32)
    sela[0:64, 0:64] = np.eye(64)
    cb16[:, C_SELA:C_SELA + 128] = sela
    selb = np.zeros((128, 128), f32)
    selb[0:64, 64:128] = np.eye(64)
    cb16[:, C_SELB:C_SELB + 128] = selb
    peb = np.zeros((128, 1024), f32)
    peb[0:64, 0:N] = (_pos_encoding_np(N, D) + conv_b[None, :]).T
    cb16[:, C_PEB:C_PEB + 1024] = peb
    dbl = lambda a: np.concatenate([a, a], axis=0)
    cb16[:, C_QWT:C_QWT + 64] = dbl(qw.T)
    cb16[0:64, C_KW:C_KW + 64] = kw
    cb16[:, C_VWT:C_VWT + 64] = dbl(vw.T)
    cb16[0:64, C_OWT:C_OWT + 64] = ow.T
    cb16[0:64, C_SELAB] = 1.0
    cb16[64:128, C_SELAB + 1] = 1.0
    cb16[:, C_ONEC] = 1.0
    cb16[:, C_SKC] = dbl(kw.sum(1)[:, None])[:, 0]
    cb16[0:64, C_RWT:C_RWT + E] = router_w.T
    cb16[0:64, C_WEXP:C_WEXP + E * D] = \
        expert_w.transpose(2, 0, 1).reshape(D, E * D)
    cb16[64, C_WEXP:C_WEXP + E * D] = expert_b.reshape(E * D)
    cb16[0:64, C_MOWT:C_MOWT + 64] = moe_out_w.T
    cb16[0:64, C_OUTWT:C_OUTWT + PRED] = out_w.T
    cb16[0, C_OWSN:C_OWSN + PRED] = -out_w.sum(1)
    cb16[0:4, C_ID4:C_ID4 + 4] = np.eye(4)

    cb32 = np.zeros((128, NC32), f32)
    cb32[0:64, Z_SQC] = qw.sum(1)
    cb32[0:64, Z_SVC] = vw.sum(1)
    cb32[:, Z_LASTM] = 1.0
    cb32[127, Z_LASTM] = 0.0
    cb32[126, Z_OH] = 1.0
    cb32[0, Z_ONESR:Z_ONESR + 128] = 1.0
    cb32[0:8, Z_IDZG8:Z_IDZG8 + 8] = np.diag([0.125] * 4 + [1.0] * 4)
    cb32[0:4, Z_ID4:Z_ID4 + 4] = np.eye(4)
    cb32[:, Z_ONEC] = 1.0

    cb16 = np.ascontiguousarray(cb16).astype(bf16)
    cb32 = np.ascontiguousarray(cb32)

    Xr = X.reshape(B, C, M6, 2)
    in_maps = []
    for c in range(NCORES):
        xs = Xr[c * SPC:(c + 1) * SPC]
        xd = np.concatenate([xs[:, :, :, 0], xs[:, :, :, 1]], axis=1)
        m = {"Xd": np.ascontiguousarray(xd).astype(bf16),
             "CB16": cb16, "CB32": cb32}
        in_maps.append(m)
    return in_maps


def kernel(**inputs) -> np.ndarray:
    nc = _get_nc()
    in_maps = _prep_in_maps(inputs)
    res = run_bass_kernel_spmd(nc, in_maps, core_ids=list(range(NCORES)))
    out = np.concatenate([res.results[c]["Yout"] for c in range(NCORES)],
                         axis=0)
    return out.astype(np.float32)
